# revision 3
# baseline (speedup 1.0000x reference)
"""CrossAttention3D Trainium2 kernel.

Problem: B=1, C=64 channels, D=H=W=16 -> N=4096 tokens, 8 heads of dim 8.
Sharding: one head per NeuronCore (8 cores). Inputs are replicated
(x tensors) or head-sliced (weights); each core computes its head's full
attention plus its partial contribution to the output projection; the host
sums the 8 partials.

Math per core h:
  x' = [x; 1]                           # [65, N] ones-row makes biases a GEMM row
  Q_h = wq'.T @ xd'   (lhsT=wq')        # [8, N]  (includes q_b)
  K_h = wk'.T @ xm'                     # [8, N]
  V1T = xm'.T @ wv'   per 128-key chunk # [N, 9]  col 8 == 1.0 exactly
  S^T = K_h.T @ Q_h   per chunk         # [128 keys, Nq] scores transposed
  P^T = exp(S^T * hd^-0.5)              # no max-subtraction: |S*scale| < ~1 for
                                        # these input scales, exp is exact-safe
  O'  = sum_chunks V1T_c.T @ P^T_c      # [9, Nq]; row 8 = softmax denominators
  F   = O'_slice.T @ wo''               # [128q, 65]; col 64 = denominator
  out^T = F[:, :64] * (1/F[:, 64:65])   # normalize after o-proj (commutes);
                                        # o_b rides in wo'' row 8 on core 0 only
Host: out = (sum_h out^T_h).T  -> [1, 64, 16, 16, 16]
"""

import numpy as np

NH = 8
HD = 8
C = 64
N = 4096
B, D, H, W = 1, 16, 16, 16
SCALE = float(HD) ** -0.5

QB = 1024  # query block (exp granularity; [9, QB] f32 psum accum = 2 banks)
KC = 128  # key chunk (PE partition dim for S^T / PV)
NQB = N // QB
NKC = N // KC

_CACHE = {}


def _build_nc():
    import concourse.tile as tile
    from concourse import bacc, mybir
    from concourse.bass import ts, ds

    f32 = mybir.dt.float32
    bf16 = mybir.dt.bfloat16

    nc = bacc.Bacc("TRN2", debug=False)

    xd1 = nc.dram_tensor("xd1", [C + 1, N], f32, kind="ExternalInput").ap()
    xm1 = nc.dram_tensor("xm1", [C + 1, N], f32, kind="ExternalInput").ap()
    wq = nc.dram_tensor("wq", [C + 1, HD], f32, kind="ExternalInput").ap()
    wk = nc.dram_tensor("wk", [C + 1, HD], f32, kind="ExternalInput").ap()
    wv = nc.dram_tensor("wv", [C + 1, HD + 1], f32, kind="ExternalInput").ap()
    wo = nc.dram_tensor("wo", [HD + 1, C + 1], f32, kind="ExternalInput").ap()
    outT = nc.dram_tensor("outT", [N, C], f32, kind="ExternalOutput").ap()

    with tile.TileContext(nc) as tc:
        with (
            tc.tile_pool(name="singles", bufs=1) as singles,
            tc.tile_pool(name="work", bufs=3) as work,
            tc.tile_pool(name="osb", bufs=2) as osb,
            tc.tile_pool(name="ps_s", bufs=2, space="PSUM") as ps_s_pool,
            tc.tile_pool(name="ps_o", bufs=1, space="PSUM") as ps_o_pool,
            tc.tile_pool(name="ps_m", bufs=2, space="PSUM") as ps_m_pool,
        ):
            # ---- loads (split across DMA queues) ----
            s_xd1 = singles.tile([C + 1, N], f32)
            s_xm1 = singles.tile([C + 1, N], f32)
            for j in range(4):
                nc.sync.dma_start(out=s_xd1[:, ts(j, N // 4)], in_=xd1[:, ts(j, N // 4)])
                nc.sync.dma_start(out=s_xm1[:, ts(j, N // 4)], in_=xm1[:, ts(j, N // 4)])
            s_wq = singles.tile([C + 1, HD], f32)
            nc.sync.dma_start(out=s_wq, in_=wq)
            s_wk = singles.tile([C + 1, HD], f32)
            nc.sync.dma_start(out=s_wk, in_=wk)
            s_wv = singles.tile([C + 1, HD + 1], f32)
            nc.sync.dma_start(out=s_wv, in_=wv)
            s_wo = singles.tile([HD + 1, C + 1], f32)
            nc.sync.dma_start(out=s_wo, in_=wo)

            s_zero = singles.tile([128, 1], f32)
            nc.vector.memset(s_zero, 0.0)

            # ---- projections ----
            s_q = singles.tile([HD, N], bf16)
            s_k = singles.tile([HD, N], bf16)
            s_v1t = singles.tile([128, NKC, HD + 1], bf16)

            for j in range(N // 512):
                pq = ps_m_pool.tile([HD, 512], f32, tag="pm", padded_shape=[128, 512])
                nc.tensor.matmul(pq, lhsT=s_wq, rhs=s_xd1[:, ts(j, 512)], start=True, stop=True)
                nc.vector.tensor_copy(out=s_q[:, ts(j, 512)], in_=pq)
                pk = ps_m_pool.tile([HD, 512], f32, tag="pm", padded_shape=[128, 512])
                nc.tensor.matmul(pk, lhsT=s_wk, rhs=s_xm1[:, ts(j, 512)], start=True, stop=True)
                nc.vector.tensor_copy(out=s_k[:, ts(j, 512)], in_=pk)
            for ci in range(NKC):
                pv = ps_m_pool.tile([128, HD + 1], f32, tag="pm", padded_shape=[128, 512])
                nc.tensor.matmul(pv, lhsT=s_xm1[:, ts(ci, 128)], rhs=s_wv, start=True, stop=True)
                nc.vector.tensor_copy(out=s_v1t[:, ci, :], in_=pv)

            # ---- attention main loop ----
            for b in range(NQB):
                po = ps_o_pool.tile([HD + 1, QB], f32, tag="po")
                for ci in range(NKC):
                    ps = ps_s_pool.tile([128, QB], f32, tag="ps")
                    for hf in range(QB // 512):
                        nc.tensor.matmul(
                            ps[:, ts(hf, 512)],
                            lhsT=s_k[:, ts(ci, KC)],
                            rhs=s_q[:, ds(b * QB + hf * 512, 512)],
                            start=True,
                            stop=True,
                        )
                    pt = work.tile([128, QB], bf16, tag="pt")
                    nc.scalar.activation(
                        out=pt,
                        in_=ps,
                        func=mybir.ActivationFunctionType.Exp,
                        bias=s_zero,
                        scale=SCALE,
                    )
                    for hf in range(QB // 512):
                        nc.tensor.matmul(
                            po[:, ts(hf, 512)],
                            lhsT=s_v1t[:, ci, :],
                            rhs=pt[:, ts(hf, 512)],
                            start=(ci == 0),
                            stop=(ci == NKC - 1),
                        )
                o_sb = osb.tile([HD + 1, QB], f32, tag="osb")
                nc.vector.tensor_copy(out=o_sb, in_=po)
                for g in range(QB // 128):
                    pf = ps_m_pool.tile([128, C + 1], f32, tag="pm", padded_shape=[128, 512])
                    nc.tensor.matmul(pf, lhsT=o_sb[:, ts(g, 128)], rhs=s_wo, start=True, stop=True)
                    rec = work.tile([128, 1], f32, tag="rec")
                    nc.vector.reciprocal(out=rec, in_=pf[:, C : C + 1])
                    fin = work.tile([128, C], f32, tag="fin")
                    nc.vector.tensor_scalar_mul(fin, pf[:, 0:C], rec)
                    nc.sync.dma_start(out=outT[ds(b * QB + g * 128, 128), :], in_=fin)
    nc.compile()
    return nc


def _prep_in_maps(inputs):
    dec = np.ascontiguousarray(np.asarray(inputs["decoder_features"], np.float32).reshape(C, N))
    mae = np.ascontiguousarray(np.asarray(inputs["mae_features"], np.float32).reshape(C, N))
    q_w = np.asarray(inputs["q_w"], np.float32)
    q_b = np.asarray(inputs["q_b"], np.float32)
    k_w = np.asarray(inputs["k_w"], np.float32)
    k_b = np.asarray(inputs["k_b"], np.float32)
    v_w = np.asarray(inputs["v_w"], np.float32)
    v_b = np.asarray(inputs["v_b"], np.float32)
    o_w = np.asarray(inputs["o_w"], np.float32)
    o_b = np.asarray(inputs["o_b"], np.float32)

    ones = np.ones((1, N), np.float32)
    xd1 = np.ascontiguousarray(np.concatenate([dec, ones], axis=0))
    xm1 = np.ascontiguousarray(np.concatenate([mae, ones], axis=0))

    in_maps = []
    for h in range(NH):
        sl = slice(h * HD, (h + 1) * HD)
        wq_h = np.concatenate([q_w[sl].T, q_b[sl][None, :]], axis=0)  # [65, 8]
        wk_h = np.concatenate([k_w[sl].T, k_b[sl][None, :]], axis=0)
        wv_h = np.zeros((C + 1, HD + 1), np.float32)
        wv_h[:C, :HD] = v_w[sl].T
        wv_h[C, :HD] = v_b[sl]
        wv_h[C, HD] = 1.0  # ones-row of xm1 -> column of exact 1.0 in V1T
        wo_h = np.zeros((HD + 1, C + 1), np.float32)
        wo_h[:HD, :C] = o_w[:, sl].T
        if h == 0:
            wo_h[HD, :C] = o_b  # rides on the denominator row; denominators
            # scale it by s_q, the final 1/s_q normalize restores o_b exactly
        wo_h[HD, C] = 1.0  # passes the denominator through to F[:, 64]
        in_maps.append(
            {
                "xd1": xd1,
                "xm1": xm1,
                "wq": np.ascontiguousarray(wq_h, dtype=np.float32),
                "wk": np.ascontiguousarray(wk_h, dtype=np.float32),
                "wv": wv_h,
                "wo": wo_h,
            }
        )
    return in_maps


def _run(inputs, trace=False):
    from concourse import bass_utils

    if "nc" not in _CACHE:
        _CACHE["nc"] = _build_nc()
    nc = _CACHE["nc"]
    in_maps = _prep_in_maps(inputs)
    res = bass_utils.run_bass_kernel_spmd(nc, in_maps, core_ids=list(range(NH)), trace=trace)
    acc = np.zeros((N, C), np.float64)
    for h in range(NH):
        acc += res.results[h]["outT"].astype(np.float64)
    out = np.ascontiguousarray(acc.T.astype(np.float32).reshape(B, C, D, H, W))
    return out, res


def kernel(**inputs) -> np.ndarray:
    out, _ = _run(inputs, trace=False)
    return out
